# revision 14
# baseline (speedup 1.0000x reference)
"""Trainium2 Bass kernel for linear multi-head attention (elu+1 feature map).

Math (per batch n):
  q = x_q @ Wq.T ; k = x_k @ Wk.T ; v = x_v @ Wv.T
  Q = elu(q)+1 ; K = elu(k)+1
  KV[h] = K_h.T @ v_h              (D x D per head)
  Ksum  = sum_s K[s, :]            (E)
  S[l,h] = Q_h[l] . Ksum_h ;  W = 1 / S          (eps/L ~ 1e-10, dropped)
  msg[l, h*D+dv] = (Q_h[l] @ KV[h])[dv] * W[l,h]
  out = msg @ Wm.T

Sharding: B*L = 16384 rows split into 8 chunks of 2048 (each core gets half
of one batch's sequence). Only cross-core dependency: the KV/Ksum reduction
between the two cores sharing a batch -> pairwise AllReduce of 67.6KB.

All inputs/weights are host-cast to bf16 (halves DMA traffic, enables FWL
fast weight loads); matmul accumulation stays f32 in PSUM. KV+Ksum are
computed packed per-head (16 MMs of N=33 per s-tile via col tiling, with a
ones column appended to v so Ksum rides along), S is computed directly in
128-partition broadcast form via a block-diagonal masked-Ksum lhsT so the
per-row normalizer needs no DMA broadcast.
"""

import numpy as np

B = 4
L = 4096
E = 512
H = 16
D = 32
P = 128
KT = E // P
NCORES = 8
R = (B * L) // NCORES
ST = R // P
NCHUNK = 4
CH = R // NCHUNK
CC = P * KT * 33

_CACHE = {}
LAST_EXEC_NS = None
LAST_RESULTS = None


def _build():
    import concourse.bass as bass
    import concourse.mybir as mybir
    import concourse.tile as tile
    from concourse import bacc

    f32 = mybir.dt.float32
    bf16 = mybir.dt.bfloat16
    AFT = mybir.ActivationFunctionType
    OP = mybir.AluOpType

    nc = bacc.Bacc("TRN2", target_bir_lowering=False, debug=False,
                   num_devices=NCORES)

    xq_d = nc.dram_tensor("xq", [E, R], bf16, kind="ExternalInput").ap()
    xk_d = nc.dram_tensor("xk", [E, R], bf16, kind="ExternalInput").ap()
    xv_d = nc.dram_tensor("xv", [E, R], bf16, kind="ExternalInput").ap()
    wq_d = nc.dram_tensor("wq", [E, E], bf16, kind="ExternalInput").ap()
    wk_d = nc.dram_tensor("wk", [E, E], bf16, kind="ExternalInput").ap()
    wv_d = nc.dram_tensor("wv", [E, E], bf16, kind="ExternalInput").ap()
    wm_d = nc.dram_tensor("wm", [E, E], bf16, kind="ExternalInput").ap()
    out_d = nc.dram_tensor("out", [R, E], f32, kind="ExternalOutput").ap()

    RG = [[0, 1], [2, 3], [4, 5], [6, 7]]

    # mask[e', p] = 1 where e' and p fall in the same 32-block; used to build
    # the block-diagonal Ksum lhsT that yields S already broadcast over the
    # 128 msg partitions.
    mask_np = np.zeros((P, P), np.float32)
    for j in range(4):
        mask_np[32 * j:32 * (j + 1), 32 * j:32 * (j + 1)] = 1.0
    mask_d = nc.inline_tensor(mask_np, name="bd_mask")

    with tile.TileContext(nc) as tc:

        def elu1(tpool, ps_in, out_ap, tag):
            """out = elu(x)+1 = Exp(-Relu(-x)) + max(x,0); 2 ACT + 1 DVE."""
            n = ps_in.shape[-1]
            tA = tpool.tile([P, n], f32, name=f"tA{tag}", tag=f"tA{tag}")
            tB = tpool.tile([P, n], f32, name=f"tB{tag}", tag=f"tB{tag}")
            nc.scalar.activation(tA[:], ps_in, AFT.Relu, scale=-1.0)
            nc.scalar.activation(tB[:], tA[:], AFT.Exp, scale=-1.0)
            nc.vector.scalar_tensor_tensor(
                out_ap, ps_in, 0.0, tB[:], OP.max, OP.add)

        with tc.tile_pool(name="const", bufs=1) as const, \
             tc.tile_pool(name="xq_pool", bufs=1) as xq_pool, \
             tc.tile_pool(name="qt_pool", bufs=1) as qt_pool, \
             tc.tile_pool(name="dram", bufs=1, space="DRAM") as dram:

            wq_sb = const.tile([P, KT, E], bf16)
            wk_sb = const.tile([P, KT, E], bf16)
            wv_sb = const.tile([P, KT, E], bf16)
            wm_sb = const.tile([P, KT, E], bf16)
            mask_sb = const.tile([P, P], f32)

            cc_in1 = dram.tile([CC], f32)
            cc_out1 = dram.tile([CC], f32)
            cc_in2 = dram.tile([CC], f32)
            cc_out2 = dram.tile([CC], f32)

            # =================== Phase A: k/v proj + KV/Ksum ===============
            with tc.tile_pool(name="xkv_pool", bufs=1) as xkv_pool, \
                 tc.tile_pool(name="workA", bufs=3) as workA, \
                 tc.tile_pool(name="psA", bufs=4, space="PSUM") as psA, \
                 tc.tile_pool(name="kvp", bufs=1, space="PSUM") as kvp:

                xk_sb = xkv_pool.tile([P, KT, R], bf16)
                xv_sb = xkv_pool.tile([P, KT, R], bf16)
                wv_r = wv_d.rearrange("(ko ki) n -> ki ko n", ki=P)
                wk_r = wk_d.rearrange("(ko ki) n -> ki ko n", ki=P)
                # v-side weights + first s-tile first so MM 0 unblocks after
                # ~0.6MB; then k-side; then the bulk of xv/xk. All per-ko
                # (128-descriptor) transfers — larger multi-segment DMAs
                # serialize descriptor execution and complete later.
                for ko in range(KT):
                    nc.sync.dma_start(wv_sb[:, ko], wv_r[:, ko])
                    nc.sync.dma_start(
                        xv_sb[:, ko, 0:P], xv_d[ko * P:(ko + 1) * P, 0:P])
                for ko in range(KT):
                    nc.gpsimd.dma_start(out=wk_sb[:, ko], in_=wk_r[:, ko])
                    nc.gpsimd.dma_start(
                        out=xk_sb[:, ko, 0:P],
                        in_=xk_d[ko * P:(ko + 1) * P, 0:P])
                for ko in range(KT):
                    nc.sync.dma_start(
                        xv_sb[:, ko, P:CH], xv_d[ko * P:(ko + 1) * P, P:CH])
                    nc.gpsimd.dma_start(
                        out=xk_sb[:, ko, P:CH],
                        in_=xk_d[ko * P:(ko + 1) * P, P:CH])
                for c in range(1, NCHUNK):
                    cs = slice(c * CH, (c + 1) * CH)
                    for ko in range(KT):
                        nc.sync.dma_start(
                            xv_sb[:, ko, cs], xv_d[ko * P:(ko + 1) * P, cs])
                        nc.gpsimd.dma_start(
                            out=xk_sb[:, ko, cs],
                            in_=xk_d[ko * P:(ko + 1) * P, cs])

                # q-side operands stream down the Activation HWDGE queue in
                # parallel with the sync queue
                for w_sb, w_d in ((wq_sb, wq_d), (wm_sb, wm_d)):
                    nc.scalar.dma_start(
                        w_sb[:], w_d.rearrange("(ko ki) n -> ki ko n", ki=P))
                nc.scalar.dma_start(mask_sb[:], mask_d.ap())
                xq_sb = xq_pool.tile([P, KT, R], bf16)
                for ko in range(KT):
                    nc.scalar.dma_start(
                        xq_sb[:, ko, :], xq_d[ko * P:(ko + 1) * P, :])

                # KV+Ksum accumulators: bank g holds heads 4g..4g+3, head
                # 4g+j at partitions 32j, cols 0:32 = KV, col 32 = Ksum.
                kvf_ps = [kvp.tile([P, 33], f32, name=f"kvf{g}")
                          for g in range(4)]

                # v33 double-buffer with a constant ones column at [:,h,32]
                v33_bufs = [xkv_pool.tile([P, H, 33], bf16, name=f"v33_{i}")
                            for i in range(2)]
                for t in v33_bufs:
                    nc.vector.memset(t[:, :, 32], 1.0)

                # software pipeline: KV(si-1) emitted after projections of
                # si so the PE never waits for the elu chain. The KV/Ksum
                # reduction is split into two collectives: AR#1 covers
                # s-tiles 0..SPLIT-1 and fires mid-phase-A (absorbing the
                # cross-core mesh-barrier skew while the PE still has local
                # work), AR#2 covers the remainder.
                SPLIT = 12
                kv_prev = None
                kv_stage = None
                for si in range(ST + 1):
                    if si < ST:
                        sl = slice(si * P, (si + 1) * P)
                        v_ps = psA.tile([P, E], f32, name="v_ps", tag="proj")
                        for ko in range(KT):
                            nc.tensor.matmul(
                                v_ps[:], xv_sb[:, ko, sl], wv_sb[:, ko, :],
                                start=(ko == 0), stop=(ko == KT - 1))
                        v33 = v33_bufs[si % 2]
                        nc.vector.tensor_copy(
                            v33[:, :, 0:32],
                            v_ps[:].rearrange("p (h d) -> p h d", h=H))

                        k_ps = psA.tile([P, E], f32, name="k_ps", tag="proj")
                        for ko in range(KT):
                            nc.tensor.matmul(
                                k_ps[:], xk_sb[:, ko, sl], wk_sb[:, ko, :],
                                start=(ko == 0), stop=(ko == KT - 1))
                        k_sb = workA.tile([P, E], bf16, name="k_sb")
                        elu1(workA, k_ps[:], k_sb[:], "k")
                        kv_stage = (k_sb, v33)
                    if si >= 1:
                        pk, pv = kv_prev
                        for g in range(4):
                            for j in range(4):
                                h = 4 * g + j
                                nc.tensor.matmul(
                                    kvf_ps[g][32 * j:32 * (j + 1), :],
                                    pk[:, 32 * h:32 * (h + 1)],
                                    pv[:, h, :],
                                    start=(si == 1 or si == SPLIT + 1),
                                    stop=(si == SPLIT or si == ST),
                                    tile_position=(0, 32 * j))
                    if si == SPLIT:
                        kv_h1 = workA.tile([P, KT, 33], f32, name="kv_h1",
                                           tag="kv_h1", bufs=1)
                        for g in range(4):
                            nc.vector.tensor_copy(kv_h1[:, g, :],
                                                  kvf_ps[g][:])
                        nc.sync.dma_start(
                            cc_in1[:].rearrange("(p g f) -> p g f",
                                                p=P, g=KT),
                            kv_h1[:])
                        nc.gpsimd.collective_compute(
                            "AllReduce", mybir.AluOpType.add,
                            replica_groups=RG,
                            ins=[cc_in1[:].opt()], outs=[cc_out1[:].opt()])
                    kv_prev = kv_stage

                kv_sb32 = workA.tile([P, KT, 33], f32, name="kv_sb32")
                for g in range(4):
                    nc.vector.tensor_copy(kv_sb32[:, g, :], kvf_ps[g][:])
                nc.sync.dma_start(
                    cc_in2[:].rearrange("(p g f) -> p g f", p=P, g=KT),
                    kv_sb32[:])

            # =================== second pairwise AllReduce ==================
            nc.gpsimd.collective_compute(
                "AllReduce", mybir.AluOpType.add, replica_groups=RG,
                ins=[cc_in2[:].opt()], outs=[cc_out2[:].opt()])

            # combine the two halves; land Ksum columns first so bd2 (and
            # thus S) unblocks before the KV payload finishes
            kvh1_sb = const.tile([P, KT, 33], f32)
            kvh2_sb = const.tile([P, KT, 33], f32)
            co1 = cc_out1[:].rearrange("(p g f) -> p g f", p=P, g=KT)
            co2 = cc_out2[:].rearrange("(p g f) -> p g f", p=P, g=KT)
            nc.sync.dma_start(kvh1_sb[:, :, 32:33], co1[:, :, 32:33])
            nc.sync.dma_start(kvh2_sb[:, :, 32:33], co2[:, :, 32:33])
            nc.sync.dma_start(kvh1_sb[:, :, 0:32], co1[:, :, 0:32])
            nc.sync.dma_start(kvh2_sb[:, :, 0:32], co2[:, :, 0:32])
            kv2_sb = const.tile([P, KT, 33], f32)
            nc.vector.tensor_tensor(
                kv2_sb[:, :, 32:33], kvh1_sb[:, :, 32:33],
                kvh2_sb[:, :, 32:33], OP.add)
            kv_bf = const.tile([P, KT, D], bf16)
            nc.vector.tensor_tensor(
                kv_bf[:], kvh1_sb[:, :, 0:32], kvh2_sb[:, :, 0:32], OP.add)
            # bd2[e', g, p] = Ksum[128g+e'] * (e'//32 == p//32): S matmul
            # lhsT that emits the normalizer already broadcast to the msg
            # partition layout.
            bd2_bf = const.tile([P, KT, P], bf16)
            for g in range(4):
                nc.vector.tensor_tensor(
                    bd2_bf[:, g, :], mask_sb[:],
                    kv2_sb[:, g, 32:33].to_broadcast((P, P)), OP.mult)

            # =================== Phase B ====================================
            with tc.tile_pool(name="workB", bufs=3) as workB, \
                 tc.tile_pool(name="msgp", bufs=2) as msgp, \
                 tc.tile_pool(name="wp", bufs=6) as wp:

                # q projection + elu for ALL chunks first (PE stays busy
                # while the collective completes); Q is stored bf16. A
                # dedicated deep PSUM pool lets the PE run several tiles
                # ahead of the elu chain.
                qt_sb = qt_pool.tile([P, KT, R], bf16)
                with tc.tile_pool(name="qps", bufs=6, space="PSUM") as qps:
                    for c in range(NCHUNK):
                        cs = slice(c * CH, (c + 1) * CH)
                        for no in range(KT):
                            q_ps = qps.tile([P, CH], f32, name="q_ps")
                            for ko in range(KT):
                                nc.tensor.matmul(
                                    q_ps[:],
                                    wq_sb[:, ko, no * P:(no + 1) * P],
                                    xq_sb[:, ko, cs],
                                    start=(ko == 0), stop=(ko == KT - 1))
                            elu1(workB, q_ps[:], qt_sb[:, no, cs], "q")

                with tc.tile_pool(name="sps", bufs=2, space="PSUM") as sps, \
                     tc.tile_pool(name="mps", bufs=2, space="PSUM") as mps, \
                     tc.tile_pool(name="ops", bufs=2, space="PSUM") as ops:
                    for c in range(NCHUNK):
                        cs = slice(c * CH, (c + 1) * CH)
                        # S in broadcast form + W = 1/S
                        ws = []
                        for g in range(4):
                            s_ps = sps.tile([P, CH], f32, name="s_ps")
                            nc.tensor.matmul(
                                s_ps[:], bd2_bf[:, g, :], qt_sb[:, g, cs],
                                start=True, stop=True)
                            w_sb = wp.tile([P, CH], f32, name=f"w{c}_{g}",
                                           tag=f"w{(4 * c + g) % 6}")
                            nc.vector.reciprocal_approx_fast(w_sb[:], s_ps[:])
                            ws.append(w_sb)

                        # message matmuls (diagonal 32x32 packing) + scale
                        msg_sb = msgp.tile([P, KT, CH], bf16, name="msg_sb")
                        for g in range(4):
                            m_ps = mps.tile([P, CH], f32, name="m_ps")
                            for j in range(4):
                                sl32 = slice(32 * j, 32 * (j + 1))
                                nc.tensor.matmul(
                                    m_ps[sl32, :],
                                    kv_bf[sl32, g, :],
                                    qt_sb[sl32, g, cs],
                                    start=True, stop=True,
                                    tile_position=(32 * j, 32 * j))
                            nc.vector.tensor_tensor(
                                msg_sb[:, g, :], m_ps[:], ws[g][:], OP.mult)

                        # merge projection; output DMAs alternate between
                        # the gpsimd and sync queues so issue + drain
                        # overlap
                        for lt in range(CH // P):
                            o_ps = ops.tile([P, E], f32, name="o_ps")
                            for g in range(KT):
                                nc.tensor.matmul(
                                    o_ps[:],
                                    msg_sb[:, g, lt * P:(lt + 1) * P],
                                    wm_sb[:, g, :],
                                    start=(g == 0), stop=(g == KT - 1))
                            o_sb = workB.tile([P, E], f32, name="o_sb")
                            nc.scalar.copy(o_sb[:], o_ps[:])
                            # split each output tile across both DMA queues
                            # so the final drain halves
                            orows = slice(c * CH + lt * P,
                                          c * CH + (lt + 1) * P)
                            nc.sync.dma_start(
                                out_d[orows, 0:E // 2], o_sb[:, 0:E // 2])
                            nc.gpsimd.dma_start(
                                out=out_d[orows, E // 2:E],
                                in_=o_sb[:, E // 2:E])

    nc.compile()
    return nc


def _get_nc():
    if "nc" not in _CACHE:
        _CACHE["nc"] = _build()
    return _CACHE["nc"]


def kernel(query, key, value, Wq, Wk, Wv, Wm):
    global LAST_EXEC_NS, LAST_RESULTS
    import os
    import ml_dtypes
    from concourse.bass_utils import run_bass_kernel_spmd

    bf = ml_dtypes.bfloat16
    query = np.asarray(query, dtype=np.float32)
    key = np.asarray(key, dtype=np.float32)
    value = np.asarray(value, dtype=np.float32)
    wq_t = np.ascontiguousarray(np.asarray(Wq, np.float32).T.astype(bf))
    wk_t = np.ascontiguousarray(np.asarray(Wk, np.float32).T.astype(bf))
    wv_t = np.ascontiguousarray(np.asarray(Wv, np.float32).T.astype(bf))
    wm_t = np.ascontiguousarray(np.asarray(Wm, np.float32).T.astype(bf))

    in_maps = []
    for c in range(NCORES):
        b, half = c // 2, c % 2
        rs = slice(half * R, (half + 1) * R)
        in_maps.append({
            "xq": np.ascontiguousarray(query[b, rs, :].T.astype(bf)),
            "xk": np.ascontiguousarray(key[b, rs, :].T.astype(bf)),
            "xv": np.ascontiguousarray(value[b, rs, :].T.astype(bf)),
            "wq": wq_t, "wk": wk_t, "wv": wv_t, "wm": wm_t,
        })

    nc = _get_nc()
    trace = bool(int(os.environ.get("KERNEL_TRACE", "0")))
    res = run_bass_kernel_spmd(nc, in_maps, core_ids=list(range(NCORES)),
                               trace=trace)
    LAST_EXEC_NS = res.exec_time_ns
    LAST_RESULTS = res

    out = np.empty((B, L, E), dtype=np.float32)
    for c in range(NCORES):
        b, half = c // 2, c % 2
        out[b, half * R:(half + 1) * R, :] = res.results[c]["out"]
    return out


# revision 16
# speedup vs baseline: 1.2336x; 1.2336x over previous
"""Trainium2 Bass kernel for linear multi-head attention (elu+1 feature map).

Math (per batch n):
  q = x_q @ Wq.T ; k = x_k @ Wk.T ; v = x_v @ Wv.T
  Q = elu(q)+1 ; K = elu(k)+1
  KV[h] = K_h.T @ v_h              (D x D per head)
  Ksum  = sum_s K[s, :]            (E)
  S[l,h] = Q_h[l] . Ksum_h ;  W = 1 / S          (eps/L ~ 1e-10, dropped)
  msg[l, h*D+dv] = (Q_h[l] @ KV[h])[dv] * W[l,h]
  out = msg @ Wm.T

Sharding: B*L = 16384 rows split into 8 chunks of 2048 (each core gets half
of one batch's sequence). Only cross-core dependency: the KV/Ksum reduction
between the two cores sharing a batch -> pairwise AllReduce of 67.6KB.

All inputs/weights are host-cast to bf16 (halves DMA traffic, enables FWL
fast weight loads); matmul accumulation stays f32 in PSUM. KV+Ksum are
computed packed per-head (16 MMs of N=33 per s-tile via col tiling, with a
ones column appended to v so Ksum rides along), S is computed directly in
128-partition broadcast form via a block-diagonal masked-Ksum lhsT so the
per-row normalizer needs no DMA broadcast.
"""

import numpy as np

B = 4
L = 4096
E = 512
H = 16
D = 32
P = 128
KT = E // P
NCORES = 8
R = (B * L) // NCORES
ST = R // P
NCHUNK = 4
CH = R // NCHUNK
CC = P * KT * 33

_CACHE = {}
LAST_EXEC_NS = None
LAST_RESULTS = None


def _build():
    import concourse.bass as bass
    import concourse.mybir as mybir
    import concourse.tile as tile
    from concourse import bacc

    f32 = mybir.dt.float32
    bf16 = mybir.dt.bfloat16
    AFT = mybir.ActivationFunctionType
    OP = mybir.AluOpType

    nc = bacc.Bacc("TRN2", target_bir_lowering=False, debug=False,
                   num_devices=NCORES)

    xq_d = nc.dram_tensor("xq", [E, R], bf16, kind="ExternalInput").ap()
    xk_d = nc.dram_tensor("xk", [E, R], bf16, kind="ExternalInput").ap()
    xv_d = nc.dram_tensor("xv", [E, R], bf16, kind="ExternalInput").ap()
    wq_d = nc.dram_tensor("wq", [E, E], bf16, kind="ExternalInput").ap()
    wk_d = nc.dram_tensor("wk", [E, E], bf16, kind="ExternalInput").ap()
    wv_d = nc.dram_tensor("wv", [E, E], bf16, kind="ExternalInput").ap()
    wm_d = nc.dram_tensor("wm", [E, E], bf16, kind="ExternalInput").ap()
    out_d = nc.dram_tensor("out", [R, E], bf16,
                           kind="ExternalOutput").ap()

    RG = [[0, 1], [2, 3], [4, 5], [6, 7]]

    # mask[e', p] = 1 where e' and p fall in the same 32-block; used to build
    # the block-diagonal Ksum lhsT that yields S already broadcast over the
    # 128 msg partitions.
    mask_np = np.zeros((P, P), np.float32)
    for j in range(4):
        mask_np[32 * j:32 * (j + 1), 32 * j:32 * (j + 1)] = 1.0
    mask_d = nc.inline_tensor(mask_np, name="bd_mask")

    with tile.TileContext(nc) as tc:

        def elu1(tpool, ps_in, out_ap, tag):
            """out = elu(x)+1 = Exp(-Relu(-x)) + max(x,0); 2 ACT + 1 DVE."""
            n = ps_in.shape[-1]
            tA = tpool.tile([P, n], f32, name=f"tA{tag}", tag=f"tA{tag}")
            tB = tpool.tile([P, n], f32, name=f"tB{tag}", tag=f"tB{tag}")
            nc.scalar.activation(tA[:], ps_in, AFT.Relu, scale=-1.0)
            nc.scalar.activation(tB[:], tA[:], AFT.Exp, scale=-1.0)
            nc.vector.scalar_tensor_tensor(
                out_ap, ps_in, 0.0, tB[:], OP.max, OP.add)

        with tc.tile_pool(name="const", bufs=1) as const, \
             tc.tile_pool(name="xq_pool", bufs=1) as xq_pool, \
             tc.tile_pool(name="qt_pool", bufs=1) as qt_pool, \
             tc.tile_pool(name="dram", bufs=1, space="DRAM") as dram:

            wq_sb = const.tile([P, KT, E], bf16)
            wk_sb = const.tile([P, KT, E], bf16)
            wv_sb = const.tile([P, KT, E], bf16)
            wm_sb = const.tile([P, KT, E], bf16)
            mask_sb = const.tile([P, P], f32)

            cc_in = dram.tile([CC], f32)
            cc_out = dram.tile([CC], f32)

            # =================== Phase A: k/v proj + KV/Ksum ===============
            with tc.tile_pool(name="xkv_pool", bufs=1) as xkv_pool, \
                 tc.tile_pool(name="workA", bufs=3) as workA, \
                 tc.tile_pool(name="psA", bufs=4, space="PSUM") as psA, \
                 tc.tile_pool(name="kvp", bufs=1, space="PSUM") as kvp:

                xk_sb = xkv_pool.tile([P, KT, R], bf16)
                xv_sb = xkv_pool.tile([P, KT, R], bf16)
                wv_r = wv_d.rearrange("(ko ki) n -> ki ko n", ki=P)
                wk_r = wk_d.rearrange("(ko ki) n -> ki ko n", ki=P)
                # v-side weights + first s-tile first so MM 0 unblocks after
                # ~0.6MB; then k-side; then the bulk of xv/xk. All per-ko
                # (128-descriptor) transfers — larger multi-segment DMAs
                # serialize descriptor execution and complete later.
                for ko in range(KT):
                    nc.sync.dma_start(wv_sb[:, ko], wv_r[:, ko])
                    nc.sync.dma_start(
                        xv_sb[:, ko, 0:P], xv_d[ko * P:(ko + 1) * P, 0:P])
                for ko in range(KT):
                    nc.gpsimd.dma_start(out=wk_sb[:, ko], in_=wk_r[:, ko])
                    nc.gpsimd.dma_start(
                        out=xk_sb[:, ko, 0:P],
                        in_=xk_d[ko * P:(ko + 1) * P, 0:P])
                for ko in range(KT):
                    nc.sync.dma_start(
                        xv_sb[:, ko, P:CH], xv_d[ko * P:(ko + 1) * P, P:CH])
                    nc.gpsimd.dma_start(
                        out=xk_sb[:, ko, P:CH],
                        in_=xk_d[ko * P:(ko + 1) * P, P:CH])
                for c in range(1, NCHUNK):
                    cs = slice(c * CH, (c + 1) * CH)
                    for ko in range(KT):
                        nc.sync.dma_start(
                            xv_sb[:, ko, cs], xv_d[ko * P:(ko + 1) * P, cs])
                        nc.gpsimd.dma_start(
                            out=xk_sb[:, ko, cs],
                            in_=xk_d[ko * P:(ko + 1) * P, cs])

                # q-side operands stream down the Activation HWDGE queue in
                # parallel with the sync queue
                for w_sb, w_d in ((wq_sb, wq_d), (wm_sb, wm_d)):
                    nc.scalar.dma_start(
                        w_sb[:], w_d.rearrange("(ko ki) n -> ki ko n", ki=P))
                nc.scalar.dma_start(mask_sb[:], mask_d.ap())
                xq_sb = xq_pool.tile([P, KT, R], bf16)
                for ko in range(KT):
                    nc.scalar.dma_start(
                        xq_sb[:, ko, :], xq_d[ko * P:(ko + 1) * P, :])

                # KV+Ksum accumulators: bank g holds heads 4g..4g+3, head
                # 4g+j at partitions 32j, cols 0:32 = KV, col 32 = Ksum.
                kvf_ps = [kvp.tile([P, 33], f32, name=f"kvf{g}")
                          for g in range(4)]

                # v33 double-buffer with a constant ones column at [:,h,32]
                v33_bufs = [xkv_pool.tile([P, H, 33], bf16, name=f"v33_{i}")
                            for i in range(2)]
                for t in v33_bufs:
                    nc.vector.memset(t[:, :, 32], 1.0)

                # software pipeline: KV(si-1) emitted after projections of
                # si so the PE never waits for the elu chain
                kv_prev = None
                kv_stage = None
                for si in range(ST + 1):
                    if si < ST:
                        sl = slice(si * P, (si + 1) * P)
                        v_ps = psA.tile([P, E], f32, name="v_ps", tag="proj")
                        for ko in range(KT):
                            nc.tensor.matmul(
                                v_ps[:], xv_sb[:, ko, sl], wv_sb[:, ko, :],
                                start=(ko == 0), stop=(ko == KT - 1))
                        v33 = v33_bufs[si % 2]
                        nc.vector.tensor_copy(
                            v33[:, :, 0:32],
                            v_ps[:].rearrange("p (h d) -> p h d", h=H))

                        k_ps = psA.tile([P, E], f32, name="k_ps", tag="proj")
                        for ko in range(KT):
                            nc.tensor.matmul(
                                k_ps[:], xk_sb[:, ko, sl], wk_sb[:, ko, :],
                                start=(ko == 0), stop=(ko == KT - 1))
                        k_sb = workA.tile([P, E], bf16, name="k_sb")
                        elu1(workA, k_ps[:], k_sb[:], "k")
                        kv_stage = (k_sb, v33)
                    if si >= 1:
                        pk, pv = kv_prev
                        for g in range(4):
                            for j in range(4):
                                h = 4 * g + j
                                nc.tensor.matmul(
                                    kvf_ps[g][32 * j:32 * (j + 1), :],
                                    pk[:, 32 * h:32 * (h + 1)],
                                    pv[:, h, :],
                                    start=(si == 1), stop=(si == ST),
                                    tile_position=(0, 32 * j))
                    kv_prev = kv_stage

                kv_sb32 = workA.tile([P, KT, 33], f32, name="kv_sb32")
                for g in range(4):
                    nc.vector.tensor_copy(kv_sb32[:, g, :], kvf_ps[g][:])
                nc.sync.dma_start(
                    cc_in[:].rearrange("(p g f) -> p g f", p=P, g=KT),
                    kv_sb32[:])

            # =================== pairwise AllReduce =========================
            nc.gpsimd.collective_compute(
                "AllReduce", mybir.AluOpType.add, replica_groups=RG,
                ins=[cc_in[:].opt()], outs=[cc_out[:].opt()])

            # land the Ksum column first so bd2 (and thus S) unblocks before
            # the KV payload finishes
            kv2_sb = const.tile([P, KT, 33], f32)
            cc_out_r = cc_out[:].rearrange("(p g f) -> p g f", p=P, g=KT)
            nc.sync.dma_start(kv2_sb[:, :, 32:33], cc_out_r[:, :, 32:33])
            nc.sync.dma_start(kv2_sb[:, :, 0:32], cc_out_r[:, :, 0:32])
            kv_bf = const.tile([P, KT, D], bf16)
            nc.vector.tensor_copy(kv_bf[:], kv2_sb[:, :, 0:32])
            # bd2[e', g, p] = Ksum[128g+e'] * (e'//32 == p//32): S matmul
            # lhsT that emits the normalizer already broadcast to the msg
            # partition layout.
            bd2_bf = const.tile([P, KT, P], bf16)
            for g in range(4):
                nc.vector.tensor_tensor(
                    bd2_bf[:, g, :], mask_sb[:],
                    kv2_sb[:, g, 32:33].to_broadcast((P, P)), OP.mult)

            # =================== Phase B ====================================
            with tc.tile_pool(name="workB", bufs=3) as workB, \
                 tc.tile_pool(name="msgp", bufs=2) as msgp, \
                 tc.tile_pool(name="wp", bufs=6) as wp:

                # q projection + elu for ALL chunks first (PE stays busy
                # while the collective completes); Q is stored bf16. A
                # dedicated deep PSUM pool lets the PE run several tiles
                # ahead of the elu chain.
                qt_sb = qt_pool.tile([P, KT, R], bf16)
                with tc.tile_pool(name="qps", bufs=6, space="PSUM") as qps:
                    for c in range(NCHUNK):
                        cs = slice(c * CH, (c + 1) * CH)
                        for no in range(KT):
                            q_ps = qps.tile([P, CH], f32, name="q_ps")
                            for ko in range(KT):
                                nc.tensor.matmul(
                                    q_ps[:],
                                    wq_sb[:, ko, no * P:(no + 1) * P],
                                    xq_sb[:, ko, cs],
                                    start=(ko == 0), stop=(ko == KT - 1))
                            elu1(workB, q_ps[:], qt_sb[:, no, cs], "q")

                with tc.tile_pool(name="sps", bufs=2, space="PSUM") as sps, \
                     tc.tile_pool(name="mps", bufs=2, space="PSUM") as mps, \
                     tc.tile_pool(name="ops", bufs=2, space="PSUM") as ops:
                    for c in range(NCHUNK):
                        cs = slice(c * CH, (c + 1) * CH)
                        # S in broadcast form + W = 1/S
                        ws = []
                        for g in range(4):
                            s_ps = sps.tile([P, CH], f32, name="s_ps")
                            nc.tensor.matmul(
                                s_ps[:], bd2_bf[:, g, :], qt_sb[:, g, cs],
                                start=True, stop=True)
                            w_sb = wp.tile([P, CH], f32, name=f"w{c}_{g}",
                                           tag=f"w{(4 * c + g) % 6}")
                            nc.vector.reciprocal_approx_fast(w_sb[:], s_ps[:])
                            ws.append(w_sb)

                        # message matmuls (diagonal 32x32 packing) + scale
                        msg_sb = msgp.tile([P, KT, CH], bf16, name="msg_sb")
                        for g in range(4):
                            m_ps = mps.tile([P, CH], f32, name="m_ps")
                            for j in range(4):
                                sl32 = slice(32 * j, 32 * (j + 1))
                                nc.tensor.matmul(
                                    m_ps[sl32, :],
                                    kv_bf[sl32, g, :],
                                    qt_sb[sl32, g, cs],
                                    start=True, stop=True,
                                    tile_position=(32 * j, 32 * j))
                            nc.vector.tensor_tensor(
                                msg_sb[:, g, :], m_ps[:], ws[g][:], OP.mult)

                        # merge projection; output DMAs alternate between
                        # the gpsimd and sync queues so issue + drain
                        # overlap
                        for lt in range(CH // P):
                            o_ps = ops.tile([P, E], f32, name="o_ps")
                            for g in range(KT):
                                nc.tensor.matmul(
                                    o_ps[:],
                                    msg_sb[:, g, lt * P:(lt + 1) * P],
                                    wm_sb[:, g, :],
                                    start=(g == 0), stop=(g == KT - 1))
                            o_sb = workB.tile([P, E], bf16, name="o_sb")
                            nc.scalar.copy(o_sb[:], o_ps[:])
                            # split each output tile across both DMA queues
                            # so the final drain halves
                            orows = slice(c * CH + lt * P,
                                          c * CH + (lt + 1) * P)
                            nc.sync.dma_start(
                                out_d[orows, 0:E // 2], o_sb[:, 0:E // 2])
                            nc.gpsimd.dma_start(
                                out=out_d[orows, E // 2:E],
                                in_=o_sb[:, E // 2:E])

    nc.compile()
    return nc


def _get_nc():
    if "nc" not in _CACHE:
        _CACHE["nc"] = _build()
    return _CACHE["nc"]


def kernel(query, key, value, Wq, Wk, Wv, Wm):
    global LAST_EXEC_NS, LAST_RESULTS
    import os
    import ml_dtypes
    from concourse.bass_utils import run_bass_kernel_spmd

    bf = ml_dtypes.bfloat16
    query = np.asarray(query, dtype=np.float32)
    key = np.asarray(key, dtype=np.float32)
    value = np.asarray(value, dtype=np.float32)
    wq_t = np.ascontiguousarray(np.asarray(Wq, np.float32).T.astype(bf))
    wk_t = np.ascontiguousarray(np.asarray(Wk, np.float32).T.astype(bf))
    wv_t = np.ascontiguousarray(np.asarray(Wv, np.float32).T.astype(bf))
    wm_t = np.ascontiguousarray(np.asarray(Wm, np.float32).T.astype(bf))

    in_maps = []
    for c in range(NCORES):
        b, half = c // 2, c % 2
        rs = slice(half * R, (half + 1) * R)
        in_maps.append({
            "xq": np.ascontiguousarray(query[b, rs, :].T.astype(bf)),
            "xk": np.ascontiguousarray(key[b, rs, :].T.astype(bf)),
            "xv": np.ascontiguousarray(value[b, rs, :].T.astype(bf)),
            "wq": wq_t, "wk": wk_t, "wv": wv_t, "wm": wm_t,
        })

    nc = _get_nc()
    trace = bool(int(os.environ.get("KERNEL_TRACE", "0")))
    res = run_bass_kernel_spmd(nc, in_maps, core_ids=list(range(NCORES)),
                               trace=trace)
    LAST_EXEC_NS = res.exec_time_ns
    LAST_RESULTS = res

    out = np.empty((B, L, E), dtype=np.float32)
    for c in range(NCORES):
        b, half = c // 2, c % 2
        out[b, half * R:(half + 1) * R, :] = res.results[c]["out"].astype(
            np.float32)
    return out
